# revision 49
# baseline (speedup 1.0000x reference)
"""Trainium2 Bass kernel for nn_Mlp_7_Layer (moe_routing).

Reference semantics: 10-tower MLP over embedded features, but the final
masked select only ever reads towers 0-4:
  col0[b] = tower[dom[b]], col1[b] = tower[{0:2,1:3,2:2,3:3,4:4}[dom[b]]]
where dom = x[15]. Towers 5-9 are dead. Also x values are in 0..4, so the
embedding lookup touches only rows 0..4 of each table, which lets layer 0
collapse: emb @ W0 == onehot(x)[B,80] @ A where A = tables[:, :5] @ W0
(K drops from 1024 to 80).

Strategy: expert-routed. Host groups (element, tower) tasks by tower,
packs them into 8 single-tower bins of R rows (R = smallest multiple of
128 with sum_t ceil(n_t / R) <= 8), runs one SPMD Bass program on the 8
cores (each core gets its bin's gathered x columns + its tower's weights),
then scatters the per-task sigmoid outputs back into the [8192, 2] result.

Device kernel per core: batch-major activations in chunks of 128 rows;
matmuls on PE (float32r by default), LayerNorm stats via bn_stats/bn_aggr
on DVE, fused relu((h - mu) * rstd) epilogue on the scalar engine, PE
transposes between layers, head2 as a DVE dot-reduce, sigmoid batched once
at the end (avoids ACT table thrash).
"""

import sys

for _p in ("/opt/trn_rl_repo",):
    if _p not in sys.path:
        sys.path.insert(0, _p)

import numpy as np
from contextlib import ExitStack

import concourse.bass as bass
import concourse.tile as tile
import concourse.mybir as mybir
from concourse import bacc
from concourse import bass_utils

# ---------------------------------------------------------------- constants
N_FIELDS = 16
EMBED = 64
DOMAIN_NUM = 5
BATCH = 8192
FCN = [N_FIELDS * EMBED, 1024, 512, 256, 128]
D_HEAD = FCN[-1]
EPS = 1e-5
P = 128

F32 = mybir.dt.float32
F32R = mybir.dt.float32r

# col1 tower for each domain value
COL1_TOWER = np.array([2, 3, 2, 3, 4], dtype=np.int64)

# Layer dims: (D_in, D_out) for the dense layers after the folded L0.
LAYER_DIMS = [(1024, 512), (512, 256), (256, 128), (128, 128)]  # L1..L3, head1

MM_DT = F32R  # precision mode for matmuls
SIM_SAFE = False  # dev: avoid ops CoreSim can't interpret
RUN_KWARGS = {}  # test harness can set {"trace": True} for NTFF profiling
LAST_RESULTS = None


# ---------------------------------------------------------------- device IR


def _ln_stats(tc, pools, ps_groups, mvg, ci):
    """bn stats+aggregate for one chunk's psum groups -> mvg[:, ci, :]."""
    nc = tc.nc
    small = pools["small"]
    n_sub = len(ps_groups)
    stats = small.tile([P, n_sub, 6], F32, tag="stats")
    for s, (ps, w) in enumerate(ps_groups):
        nc.vector.bn_stats(out=stats[:, s, :], in_=ps)
    nc.vector.bn_aggr(out=mvg[:, ci, :], in_=stats)


def _group_scale(tc, pools, mvg, glen, mm_dt):
    """Batched rstd / (-mu*rstd) for a whole group: [128, glen, 1] each."""
    nc = tc.nc
    small = pools["small"]
    rstdg = small.tile([P, glen, 1], F32, tag="rstdg")
    if mm_dt == F32R and not SIM_SAFE:
        nc.scalar.activation(out=rstdg, in_=mvg[:, 0:glen, 1:2],
                             func=mybir.ActivationFunctionType.Abs_reciprocal_sqrt,
                             bias=pools["eps"], scale=1.0)
    else:
        stdg = small.tile([P, glen, 1], F32, tag="stdg")
        nc.scalar.activation(out=stdg, in_=mvg[:, 0:glen, 1:2],
                             func=mybir.ActivationFunctionType.Sqrt,
                             bias=pools["eps"], scale=1.0)
        nc.vector.reciprocal(out=rstdg, in_=stdg)
    nmrg = small.tile([P, glen, 1], F32, tag="nmrg")
    if glen == 1:
        nc.vector.tensor_scalar(out=nmrg, in0=mvg[:, 0:1, 0:1],
                                scalar1=rstdg[:, 0, :], scalar2=-1.0,
                                op0=mybir.AluOpType.mult,
                                op1=mybir.AluOpType.mult)
    else:
        nc.vector.tensor_mul(out=nmrg, in0=mvg[:, 0:glen, 0:1], in1=rstdg)
        nc.vector.tensor_scalar_mul(nmrg, nmrg, -1.0)
    return rstdg, nmrg


def _relu_epilogue(tc, pools, ps_groups, d_out, mm_dt, rstdg, nmrg, ci):
    """relu((ps - mu) * rstd) -> SBUF [128, d_out] tile (one chunk)."""
    nc = tc.nc
    apool = pools["act0"] if d_out == FCN[1] else pools["acth"]
    h_sb = apool.tile([P, d_out], mm_dt, tag=f"act{d_out}")
    j0 = 0
    for ps, wdt in ps_groups:
        nc.scalar.activation(out=h_sb[:, j0:j0 + wdt], in_=ps,
                             func=mybir.ActivationFunctionType.Relu,
                             bias=nmrg[:, ci, :], scale=rstdg[:, ci, :])
        j0 += wdt
    return h_sb


def _act_relu_copy(nc, out, in_):
    """Copy of non-negative data on ACT without an activation-table swap."""
    nc.scalar.activation(out=out, in_=in_,
                         func=mybir.ActivationFunctionType.Relu,
                         bias=0.0, scale=1.0)


def _transpose_tiles(tc, pools, h_sb, d, ident, mm_dt):
    """h_sb [128, d] (post-relu) -> hT SBUF tile [128, d//128, 128].

    PSUM evacuation copies alternate between DVE and ACT (as a relu-copy,
    which is the identity on post-relu data and avoids a table swap).
    """
    nc = tc.nc
    k_tiles = d // P
    hT = pools["act"].tile([P, k_tiles, P], mm_dt, tag=f"hT{d}")
    for gi, g0 in enumerate(range(0, k_tiles, 4)):
        gn = min(4, k_tiles - g0)
        ps = pools["ps_tr"].tile([P, 4, P], mm_dt, tag="ps_tr")
        for k in range(gn):
            nc.tensor.transpose(ps[:, k, :],
                                h_sb[:, (g0 + k) * P:(g0 + k + 1) * P], ident)
        use_act = (d == 1024 and gi % 2 == 0) or d in (256, 128)
        if use_act:
            _act_relu_copy(nc, hT[:, g0:g0 + gn, :], ps[:, 0:gn, :])
        else:
            nc.vector.tensor_copy(out=hT[:, g0:g0 + gn, :], in_=ps[:, 0:gn, :])
    return hT


def build_program(R, mm_dt=F32R):
    """Build the SPMD Bass program for one bin of R rows (R % 128 == 0)."""
    nchunk = R // P
    nc = bacc.Bacc("TRN2", target_bir_lowering=False, debug=False,
                   num_devices=8)

    # ---- DRAM I/O
    xg_d = nc.dram_tensor("xg", [N_FIELDS * DOMAIN_NUM, R], F32,
                          kind="ExternalInput").ap()
    iota_d = nc.dram_tensor("iota80", [80, 1], F32, kind="ExternalInput").ap()
    ident_d = nc.dram_tensor("ident", [P, P], F32, kind="ExternalInput").ap()
    a_d = nc.dram_tensor("A", [80, FCN[1]], F32, kind="ExternalInput").ap()
    w_ds = []
    for i, (d_in, d_out) in enumerate(LAYER_DIMS):
        w_ds.append(nc.dram_tensor(f"W{i+1}", [P, d_in // P, d_out], F32,
                                   kind="ExternalInput").ap())
    hw2_d = nc.dram_tensor("hW2", [P, 1], F32, kind="ExternalInput").ap()
    hb2_d = nc.dram_tensor("hb2", [1, 1], F32, kind="ExternalInput").ap()
    out_d = nc.dram_tensor("out", [R, 1], F32, kind="ExternalOutput").ap()

    with tile.TileContext(nc) as tc:
        with ExitStack() as ctx:
            pools = {}
            pools["w"] = ctx.enter_context(tc.tile_pool(name="w", bufs=1))
            pools["stage"] = ctx.enter_context(tc.tile_pool(name="stage", bufs=2))
            pools["act"] = ctx.enter_context(tc.tile_pool(name="act", bufs=6))
            pools["act0"] = ctx.enter_context(tc.tile_pool(name="act0", bufs=10))
            pools["acth"] = ctx.enter_context(tc.tile_pool(name="acth", bufs=10))
            pools["small"] = ctx.enter_context(tc.tile_pool(name="small", bufs=10))
            pools["ps_mm"] = ctx.enter_context(
                tc.tile_pool(name="ps_mm", bufs=6, space="PSUM"))
            pools["ps_tr"] = ctx.enter_context(
                tc.tile_pool(name="ps_tr", bufs=2, space="PSUM"))

            w = pools["w"]
            stage = pools["stage"]

            def load_rounded(dram_ap, shape, tag, eng=None, piece_dim=None,
                             piece=4):
                """DMA fp32 DRAM -> staging, DVE copy -> mm_dt tile.

                piece_dim: stage/copy piece-by-piece along that middle dim so
                the staging slot stays small and loads pipeline.
                """
                eng = eng or nc.sync
                t = w.tile(shape, mm_dt, tag=tag)
                if mm_dt == F32:
                    eng.dma_start(out=t, in_=dram_ap)
                    return t
                if piece_dim is None:
                    s = stage.tile(shape, F32, tag="stage")
                    eng.dma_start(out=s, in_=dram_ap)
                    nc.vector.tensor_copy(out=t, in_=s)
                    return t
                n = shape[piece_dim]
                assert piece_dim == 1 and len(shape) == 3
                for k0 in range(0, n, piece):
                    kn = min(piece, n - k0)
                    s = stage.tile([shape[0], piece, shape[2]], F32,
                                   tag="stage")
                    eng.dma_start(out=s[:, 0:kn, :],
                                  in_=dram_ap[:, k0:k0 + kn, :])
                    nc.vector.tensor_copy(out=t[:, k0:k0 + kn, :],
                                          in_=s[:, 0:kn, :])
                return t

            eps_t = w.tile([P, 1], F32, tag="eps")
            nc.vector.memset(eps_t, EPS)
            pools["eps"] = eps_t

            # one-hot built lazily per group (inside emit_l0) so the first
            # L0 matmul only waits for the first group's slice of xg.
            iota_sb = w.tile([80, 1], F32, tag="iota")
            nc.sync.dma_start(out=iota_sb, in_=iota_d)
            xg80 = w.tile([80, R], F32, tag="xg80")
            oh = w.tile([80, R], mm_dt, tag="oh")

            # A in halves so the first L0 matmul (which reads A[:, 0:512])
            # only waits for the first half
            a_sb = w.tile([80, FCN[1]], mm_dt, tag="A")
            for j0 in (0, 512):
                s = stage.tile([80, 512], F32, tag="stageA")
                nc.sync.dma_start(out=s, in_=a_d[:, j0:j0 + 512])
                nc.vector.tensor_copy(out=a_sb[:, j0:j0 + 512], in_=s)
            # bulk weights: DMA-trigger from gpsimd so SyncE stays free for
            # the startup-critical transfers; stage piece-wise to pipeline
            w_sbs = [load_rounded(w_ds[i], [P, d_in // P, d_out], f"W{i+1}",
                                  eng=nc.gpsimd,
                                  piece_dim=1 if d_in // P > 4 else None,
                                  piece=4)
                     for i, (d_in, d_out) in enumerate(LAYER_DIMS)]
            ident_sb = load_rounded(ident_d, [P, P], "ident")
            # head2 weight broadcast across partitions: w2b[p, i] = hW2[i]
            w2b = w.tile([P, P], F32, tag="w2b")
            hw2_bcast = bass.AP(tensor=hw2_d.tensor, offset=hw2_d.offset,
                                ap=[[0, P], [1, P]])
            nc.sync.dma_start(out=w2b, in_=hw2_bcast)
            hb2_sb = w.tile([P, 1], F32, tag="hb2")
            nc.sync.dma_start(out=hb2_sb, in_=hb2_d.to_broadcast((P, 1)))

            # persistent pre-sigmoid logits [128, nchunk]
            s_sb = w.tile([P, nchunk], F32, tag="s")

            # Software pipeline: process chunks in groups of ~G, emitting each
            # phase for the whole group so PE gets dense back-to-back work
            # (keeps HAM warm) while ACT/DVE epilogues overlap. The next
            # group's L0 matmuls are emitted before the previous group's tail
            # layers so PE never starves at group boundaries.
            # Group sizes ramp up then down (e.g. 13 -> [2,4,4,2,1]): small
            # first group fills the pipeline fast, small last groups drain it.
            G = 4
            sizes = []
            rem = nchunk
            if rem > 2 * G:
                sizes.append(2)
                rem -= 2
            while rem > 3:
                s = min(G, rem - 3) if rem - G < 3 else G
                s = min(G, max(1, s))
                sizes.append(s)
                rem -= s
            if rem >= 2:
                sizes.append(rem - 1)
                rem = 1
            if rem:
                sizes.append(rem)
            bounds = [0]
            for s in sizes:
                bounds.append(bounds[-1] + s)
            groups = [range(bounds[i], bounds[i + 1])
                      for i in range(len(sizes))]
            n_groups = len(groups)

            def emit_l0(grp):
                cols = slice(grp.start * P, grp.stop * P)
                nc.sync.dma_start(out=xg80[:, cols], in_=xg_d[:, cols])
                nc.vector.tensor_scalar(out=oh[:, cols], in0=xg80[:, cols],
                                        scalar1=iota_sb, scalar2=None,
                                        op0=mybir.AluOpType.is_equal)
                hs = {}
                for c in grp:
                    cs = slice(c * P, (c + 1) * P)
                    mvg = pools["small"].tile([P, 1, 2], F32, tag="mvg")
                    g = []
                    for j0 in range(0, FCN[1], 512):
                        ps = pools["ps_mm"].tile([P, 512], F32, tag="ps_mm")
                        nc.tensor.matmul(ps, oh[:, cs], a_sb[:, j0:j0 + 512],
                                         start=True, stop=True)
                        g.append((ps, 512))
                    _ln_stats(tc, pools, g, mvg, 0)
                    rstdg, nmrg = _group_scale(tc, pools, mvg, 1, mm_dt)
                    hs[c] = _relu_epilogue(tc, pools, g, FCN[1], mm_dt,
                                           rstdg, nmrg, 0)
                return hs

            def emit_one_layer(grp, hs, i):
                d_in, d_out = LAYER_DIMS[i]
                hTs = {c: _transpose_tiles(tc, pools, hs[c], d_in,
                                           ident_sb, mm_dt) for c in grp}
                k_tiles = d_in // P
                hs = {}
                for c in grp:
                    mvg = pools["small"].tile([P, 1, 2], F32, tag="mvg")
                    ps = pools["ps_mm"].tile([P, 512], F32, tag="ps_mm")
                    for k in range(k_tiles):
                        nc.tensor.matmul(ps[:, 0:d_out], hTs[c][:, k, :],
                                         w_sbs[i][:, k, :],
                                         start=(k == 0),
                                         stop=(k == k_tiles - 1))
                    _ln_stats(tc, pools, [(ps[:, 0:d_out], d_out)], mvg, 0)
                    rstdg, nmrg = _group_scale(tc, pools, mvg, 1, mm_dt)
                    hs[c] = _relu_epilogue(
                        tc, pools, [(ps[:, 0:d_out], d_out)], d_out,
                        mm_dt, rstdg, nmrg, 0)
                if i == len(LAYER_DIMS) - 1:
                    # head2 dot on DVE: s[:, c] = sum_i h[:, i] * hW2[i]
                    # (tensor_tensor_reduce crashes the device)
                    for c in grp:
                        scratch = pools["act"].tile([P, P], F32, tag="scr")
                        nc.vector.tensor_mul(out=scratch,
                                             in0=hs[c].bitcast(F32), in1=w2b)
                        nc.vector.reduce_sum(out=s_sb[:, c:c + 1], in_=scratch,
                                             axis=mybir.AxisListType.X)
                return hs

            # 2D wavefront over (group, stage): stage 0 = L0, stages 1..4 =
            # dense layers (stage 4 also emits the head dot). Anti-diagonal
            # emission keeps every engine fed from 2-3 groups at once and
            # shrinks the pipeline tail to a single stage.
            n_stage = 1 + len(LAYER_DIMS)
            state = {}
            for diag in range(n_groups + n_stage - 1):
                for g in range(min(diag, n_groups - 1), -1, -1):
                    s = diag - g
                    if s < 0 or s >= n_stage:
                        continue
                    if s == 0:
                        state[g] = emit_l0(groups[g])
                    else:
                        state[g] = emit_one_layer(groups[g], state[g], s - 1)

            # sigmoid once, then one strided DMA back to [R, 1]
            o_sb = w.tile([P, nchunk], F32, tag="o")
            nc.scalar.activation(out=o_sb, in_=s_sb,
                                 func=mybir.ActivationFunctionType.Sigmoid,
                                 bias=hb2_sb, scale=1.0)
            out_ap = out_d.rearrange("(c p) o -> p (c o)", p=P)
            nc.sync.dma_start(out=out_ap, in_=o_sb)

    nc.compile()
    return nc


# ---------------------------------------------------------------- host side


def _plan_routing(dom):
    """Group (element, tower) tasks by tower; pack into 8 bins of R rows."""
    col1 = COL1_TOWER[dom]
    elems = []
    for t in range(DOMAIN_NUM):
        e = np.nonzero((dom == t) | (col1 == t))[0]
        elems.append(e)
    sizes = [len(e) for e in elems]
    R = None
    for r in range(P, BATCH * 2 + P, P):
        if sum(-(-s // r) for s in sizes if s) <= 8:
            R = r
            break
    assert R is not None
    bins = []  # (tower, element_idx_array)
    for t in range(DOMAIN_NUM):
        e = elems[t]
        for b0 in range(0, len(e), R):
            bins.append((t, e[b0:b0 + R]))
    while len(bins) < 8:
        bins.append((0, np.zeros(0, np.int64)))
    assert len(bins) <= 8
    return R, bins, col1


def _fold_A(tables, W0, t):
    """A_t [80, 1024] = stack_f tables[f,:5] @ W0[t, 64f:64f+64, :]."""
    A = np.empty((N_FIELDS * DOMAIN_NUM, FCN[1]), np.float32)
    for f in range(N_FIELDS):
        A[f * DOMAIN_NUM:(f + 1) * DOMAIN_NUM] = (
            tables[f, :DOMAIN_NUM, :].astype(np.float64)
            @ W0[t, f * EMBED:(f + 1) * EMBED, :].astype(np.float64)
        ).astype(np.float32)
    return A


def _trivial_ln(mlp_bs, mlp_gs, mlp_betas, hb1, hg, hbeta):
    ok = all(np.all(np.asarray(b) == 0) for b in mlp_bs)
    ok &= all(np.all(np.asarray(g) == 1) for g in mlp_gs)
    ok &= all(np.all(np.asarray(be) == 0) for be in mlp_betas)
    ok &= np.all(np.asarray(hb1) == 0) and np.all(np.asarray(hg) == 1)
    ok &= bool(np.all(np.asarray(hbeta) == 0))
    return ok


def kernel(x, tables, mlp_Ws, mlp_bs, mlp_gs, mlp_betas, hW1, hb1, hg,
           hbeta, hW2, hb2):
    x = np.asarray(x)
    tables = np.asarray(tables, np.float32)
    mlp_Ws = [np.asarray(w, np.float32) for w in mlp_Ws]
    hW1 = np.asarray(hW1, np.float32)
    hW2 = np.asarray(hW2, np.float32)
    hb2 = np.asarray(hb2, np.float32)

    assert _trivial_ln(mlp_bs, mlp_gs, mlp_betas, hb1, hg, hbeta), \
        "non-trivial LN affine params not supported by this build"
    assert x.min() >= 0 and x.max() < DOMAIN_NUM

    dom = np.asarray(x[N_FIELDS - 1], np.int64)
    R, bins, col1 = _plan_routing(dom)

    nc = build_program(R, MM_DT)

    iota80 = (np.arange(80) % DOMAIN_NUM).astype(np.float32).reshape(80, 1)
    ident = np.eye(P, dtype=np.float32)

    tower_cache = {}

    def tower_inputs(t):
        if t not in tower_cache:
            ws = {}
            ws["A"] = _fold_A(tables, mlp_Ws[0], t)
            chain = [mlp_Ws[1][t], mlp_Ws[2][t], mlp_Ws[3][t], hW1[t]]
            for i, wmat in enumerate(chain):
                d_in, d_out = wmat.shape
                ws[f"W{i+1}"] = np.ascontiguousarray(
                    wmat.reshape(d_in // P, P, d_out).transpose(1, 0, 2))
            ws["hW2"] = hW2[t].reshape(P, 1)
            ws["hb2"] = hb2[t].reshape(1, 1).astype(np.float32)
            tower_cache[t] = ws
        return tower_cache[t]

    in_maps = []
    for t, e in bins:
        xe = np.zeros((N_FIELDS, R), np.float32)
        if len(e):
            xe[:, :len(e)] = x[:, e].astype(np.float32)
        xe = np.repeat(xe, DOMAIN_NUM, axis=0)  # [80, R], row f*5+d = x[f]
        m = {"xg": xe, "iota80": iota80, "ident": ident}
        m.update(tower_inputs(t))
        in_maps.append(m)

    res = bass_utils.run_bass_kernel_spmd(nc, in_maps, core_ids=list(range(8)),
                                          **RUN_KWARGS)
    global LAST_RESULTS
    LAST_RESULTS = res

    out = np.zeros((BATCH, 2), np.float32)
    for (t, e), r in zip(bins, res.results):
        if not len(e):
            continue
        vals = r["out"][:len(e), 0]
        m0 = dom[e] == t
        out[e[m0], 0] = vals[m0]
        m1 = col1[e] == t
        out[e[m1], 1] = vals[m1]
    return out


# revision 50
# speedup vs baseline: 1.0299x; 1.0299x over previous
"""Trainium2 Bass kernel for nn_Mlp_7_Layer (moe_routing).

Reference semantics: 10-tower MLP over embedded features, but the final
masked select only ever reads towers 0-4:
  col0[b] = tower[dom[b]], col1[b] = tower[{0:2,1:3,2:2,3:3,4:4}[dom[b]]]
where dom = x[15]. Towers 5-9 are dead. Also x values are in 0..4, so the
embedding lookup touches only rows 0..4 of each table, which lets layer 0
collapse: emb @ W0 == onehot(x)[B,80] @ A where A = tables[:, :5] @ W0
(K drops from 1024 to 80).

Strategy: expert-routed. Host groups (element, tower) tasks by tower,
packs them into 8 single-tower bins of R rows (R = smallest multiple of
128 with sum_t ceil(n_t / R) <= 8), runs one SPMD Bass program on the 8
cores (each core gets its bin's gathered x columns + its tower's weights),
then scatters the per-task sigmoid outputs back into the [8192, 2] result.

Device kernel per core: batch-major activations in chunks of 128 rows;
matmuls on PE (float32r by default), LayerNorm stats via bn_stats/bn_aggr
on DVE, fused relu((h - mu) * rstd) epilogue on the scalar engine, PE
transposes between layers, head2 as a DVE dot-reduce, sigmoid batched once
at the end (avoids ACT table thrash).
"""

import sys

for _p in ("/opt/trn_rl_repo",):
    if _p not in sys.path:
        sys.path.insert(0, _p)

import numpy as np
from contextlib import ExitStack

import concourse.bass as bass
import concourse.tile as tile
import concourse.mybir as mybir
from concourse import bacc
from concourse import bass_utils

# ---------------------------------------------------------------- constants
N_FIELDS = 16
EMBED = 64
DOMAIN_NUM = 5
BATCH = 8192
FCN = [N_FIELDS * EMBED, 1024, 512, 256, 128]
D_HEAD = FCN[-1]
EPS = 1e-5
P = 128

F32 = mybir.dt.float32
F32R = mybir.dt.float32r

# col1 tower for each domain value
COL1_TOWER = np.array([2, 3, 2, 3, 4], dtype=np.int64)

# Layer dims: (D_in, D_out) for the dense layers after the folded L0.
LAYER_DIMS = [(1024, 512), (512, 256), (256, 128), (128, 128)]  # L1..L3, head1

MM_DT = F32R  # precision mode for matmuls
SIM_SAFE = False  # dev: avoid ops CoreSim can't interpret
RUN_KWARGS = {}  # test harness can set {"trace": True} for NTFF profiling
LAST_RESULTS = None


# ---------------------------------------------------------------- device IR


def _ln_stats(tc, pools, ps_groups, mvg, ci):
    """bn stats+aggregate for one chunk's psum groups -> mvg[:, ci, :]."""
    nc = tc.nc
    small = pools["small"]
    n_sub = len(ps_groups)
    stats = small.tile([P, n_sub, 6], F32, tag="stats")
    for s, (ps, w) in enumerate(ps_groups):
        nc.vector.bn_stats(out=stats[:, s, :], in_=ps)
    nc.vector.bn_aggr(out=mvg[:, ci, :], in_=stats)


def _group_scale(tc, pools, mvg, glen, mm_dt):
    """Batched rstd / (-mu*rstd) for a whole group: [128, glen, 1] each."""
    nc = tc.nc
    small = pools["small"]
    rstdg = small.tile([P, glen, 1], F32, tag="rstdg")
    if mm_dt == F32R and not SIM_SAFE:
        nc.scalar.activation(out=rstdg, in_=mvg[:, 0:glen, 1:2],
                             func=mybir.ActivationFunctionType.Abs_reciprocal_sqrt,
                             bias=pools["eps"], scale=1.0)
    else:
        stdg = small.tile([P, glen, 1], F32, tag="stdg")
        nc.scalar.activation(out=stdg, in_=mvg[:, 0:glen, 1:2],
                             func=mybir.ActivationFunctionType.Sqrt,
                             bias=pools["eps"], scale=1.0)
        nc.vector.reciprocal(out=rstdg, in_=stdg)
    nmrg = small.tile([P, glen, 1], F32, tag="nmrg")
    if glen == 1:
        nc.vector.tensor_scalar(out=nmrg, in0=mvg[:, 0:1, 0:1],
                                scalar1=rstdg[:, 0, :], scalar2=-1.0,
                                op0=mybir.AluOpType.mult,
                                op1=mybir.AluOpType.mult)
    else:
        nc.vector.tensor_mul(out=nmrg, in0=mvg[:, 0:glen, 0:1], in1=rstdg)
        nc.vector.tensor_scalar_mul(nmrg, nmrg, -1.0)
    return rstdg, nmrg


def _relu_epilogue(tc, pools, ps_groups, d_out, mm_dt, rstdg, nmrg, ci):
    """relu((ps - mu) * rstd) -> SBUF [128, d_out] tile (one chunk)."""
    nc = tc.nc
    apool = pools["act0"] if d_out == FCN[1] else pools["acth"]
    h_sb = apool.tile([P, d_out], mm_dt, tag=f"act{d_out}")
    j0 = 0
    for ps, wdt in ps_groups:
        nc.scalar.activation(out=h_sb[:, j0:j0 + wdt], in_=ps,
                             func=mybir.ActivationFunctionType.Relu,
                             bias=nmrg[:, ci, :], scale=rstdg[:, ci, :])
        j0 += wdt
    return h_sb


def _act_relu_copy(nc, out, in_):
    """Copy of non-negative data on ACT without an activation-table swap."""
    nc.scalar.activation(out=out, in_=in_,
                         func=mybir.ActivationFunctionType.Relu,
                         bias=0.0, scale=1.0)


def _transpose_tiles(tc, pools, h_sb, d, ident, mm_dt):
    """h_sb [128, d] (post-relu) -> hT SBUF tile [128, d//128, 128].

    PSUM evacuation copies alternate between DVE and ACT (as a relu-copy,
    which is the identity on post-relu data and avoids a table swap).
    """
    nc = tc.nc
    k_tiles = d // P
    hT = pools["act"].tile([P, k_tiles, P], mm_dt, tag=f"hT{d}")
    for gi, g0 in enumerate(range(0, k_tiles, 4)):
        gn = min(4, k_tiles - g0)
        ps = pools["ps_tr"].tile([P, 4, P], mm_dt, tag="ps_tr")
        for k in range(gn):
            nc.tensor.transpose(ps[:, k, :],
                                h_sb[:, (g0 + k) * P:(g0 + k + 1) * P], ident)
        use_act = (d == 1024 and gi % 2 == 0) or d in (256, 128)
        if use_act:
            _act_relu_copy(nc, hT[:, g0:g0 + gn, :], ps[:, 0:gn, :])
        else:
            nc.vector.tensor_copy(out=hT[:, g0:g0 + gn, :], in_=ps[:, 0:gn, :])
    return hT


def build_program(R, mm_dt=F32R):
    """Build the SPMD Bass program for one bin of R rows (R % 128 == 0)."""
    nchunk = R // P
    nc = bacc.Bacc("TRN2", target_bir_lowering=False, debug=False,
                   num_devices=8)

    # ---- DRAM I/O
    xg_d = nc.dram_tensor("xg", [N_FIELDS * DOMAIN_NUM, R], F32,
                          kind="ExternalInput").ap()
    iota_d = nc.dram_tensor("iota80", [80, 1], F32, kind="ExternalInput").ap()
    ident_d = nc.dram_tensor("ident", [P, P], F32, kind="ExternalInput").ap()
    a_d = nc.dram_tensor("A", [80, FCN[1]], F32, kind="ExternalInput").ap()
    w_ds = []
    for i, (d_in, d_out) in enumerate(LAYER_DIMS):
        w_ds.append(nc.dram_tensor(f"W{i+1}", [P, d_in // P, d_out], F32,
                                   kind="ExternalInput").ap())
    hw2_d = nc.dram_tensor("hW2", [P, 1], F32, kind="ExternalInput").ap()
    hb2_d = nc.dram_tensor("hb2", [1, 1], F32, kind="ExternalInput").ap()
    out_d = nc.dram_tensor("out", [R, 1], F32, kind="ExternalOutput").ap()

    with tile.TileContext(nc) as tc:
        with ExitStack() as ctx:
            pools = {}
            pools["w"] = ctx.enter_context(tc.tile_pool(name="w", bufs=1))
            pools["stage"] = ctx.enter_context(tc.tile_pool(name="stage", bufs=2))
            pools["act"] = ctx.enter_context(tc.tile_pool(name="act", bufs=6))
            pools["act0"] = ctx.enter_context(tc.tile_pool(name="act0", bufs=10))
            pools["acth"] = ctx.enter_context(tc.tile_pool(name="acth", bufs=10))
            pools["small"] = ctx.enter_context(tc.tile_pool(name="small", bufs=10))
            pools["ps_mm"] = ctx.enter_context(
                tc.tile_pool(name="ps_mm", bufs=6, space="PSUM"))
            pools["ps_tr"] = ctx.enter_context(
                tc.tile_pool(name="ps_tr", bufs=2, space="PSUM"))

            w = pools["w"]
            stage = pools["stage"]

            def load_rounded(dram_ap, shape, tag, eng=None, piece_dim=None,
                             piece=4):
                """DMA fp32 DRAM -> staging, DVE copy -> mm_dt tile.

                piece_dim: stage/copy piece-by-piece along that middle dim so
                the staging slot stays small and loads pipeline.
                """
                eng = eng or nc.sync
                t = w.tile(shape, mm_dt, tag=tag)
                if mm_dt == F32:
                    eng.dma_start(out=t, in_=dram_ap)
                    return t
                if piece_dim is None:
                    s = stage.tile(shape, F32, tag="stage")
                    eng.dma_start(out=s, in_=dram_ap)
                    nc.vector.tensor_copy(out=t, in_=s)
                    return t
                n = shape[piece_dim]
                assert piece_dim == 1 and len(shape) == 3
                for k0 in range(0, n, piece):
                    kn = min(piece, n - k0)
                    s = stage.tile([shape[0], piece, shape[2]], F32,
                                   tag="stage")
                    eng.dma_start(out=s[:, 0:kn, :],
                                  in_=dram_ap[:, k0:k0 + kn, :])
                    nc.vector.tensor_copy(out=t[:, k0:k0 + kn, :],
                                          in_=s[:, 0:kn, :])
                return t

            eps_t = w.tile([P, 1], F32, tag="eps")
            nc.vector.memset(eps_t, EPS)
            pools["eps"] = eps_t

            # one-hot built lazily per group (inside emit_l0) so the first
            # L0 matmul only waits for the first group's slice of xg.
            iota_sb = w.tile([80, 1], F32, tag="iota")
            nc.sync.dma_start(out=iota_sb, in_=iota_d)
            xg80 = w.tile([80, R], F32, tag="xg80")
            oh = w.tile([80, R], mm_dt, tag="oh")

            # A in halves so the first L0 matmul (which reads A[:, 0:512])
            # only waits for the first half
            a_sb = w.tile([80, FCN[1]], mm_dt, tag="A")
            for j0 in (0, 512):
                s = stage.tile([80, 512], F32, tag="stageA")
                nc.sync.dma_start(out=s, in_=a_d[:, j0:j0 + 512])
                nc.vector.tensor_copy(out=a_sb[:, j0:j0 + 512], in_=s)
            # bulk weights: DMA-trigger from gpsimd so SyncE stays free for
            # the startup-critical transfers; stage piece-wise to pipeline
            w_sbs = [load_rounded(w_ds[i], [P, d_in // P, d_out], f"W{i+1}",
                                  eng=nc.gpsimd,
                                  piece_dim=1 if d_in // P > 4 else None,
                                  piece=4)
                     for i, (d_in, d_out) in enumerate(LAYER_DIMS)]
            ident_sb = load_rounded(ident_d, [P, P], "ident")
            # head2 weight broadcast across partitions: w2b[p, i] = hW2[i]
            w2b = w.tile([P, P], F32, tag="w2b")
            hw2_bcast = bass.AP(tensor=hw2_d.tensor, offset=hw2_d.offset,
                                ap=[[0, P], [1, P]])
            nc.sync.dma_start(out=w2b, in_=hw2_bcast)
            hb2_sb = w.tile([P, 1], F32, tag="hb2")
            nc.sync.dma_start(out=hb2_sb, in_=hb2_d.to_broadcast((P, 1)))

            # persistent pre-sigmoid logits [128, nchunk]
            s_sb = w.tile([P, nchunk], F32, tag="s")

            # Software pipeline: process chunks in groups of ~G, emitting each
            # phase for the whole group so PE gets dense back-to-back work
            # (keeps HAM warm) while ACT/DVE epilogues overlap. The next
            # group's L0 matmuls are emitted before the previous group's tail
            # layers so PE never starves at group boundaries.
            # Descending group sizes (e.g. 13 -> [4,4,2,2,1]): the wavefront's
            # tail is the last group's final stage, so keep late groups small.
            G = 4
            n_groups = max(2, (nchunk + G - 1) // G + 1) if nchunk > G else 1
            wts = list(range(n_groups, 0, -1))
            tot = sum(wts)
            bounds = [0]
            acc = 0.0
            for wt in wts:
                acc += nchunk * wt / tot
                bounds.append(round(acc))
            bounds[-1] = nchunk
            groups = [range(bounds[i], bounds[i + 1]) for i in range(n_groups)
                      if bounds[i] < bounds[i + 1]]
            n_groups = len(groups)

            def emit_l0(grp):
                cols = slice(grp.start * P, grp.stop * P)
                nc.sync.dma_start(out=xg80[:, cols], in_=xg_d[:, cols])
                nc.vector.tensor_scalar(out=oh[:, cols], in0=xg80[:, cols],
                                        scalar1=iota_sb, scalar2=None,
                                        op0=mybir.AluOpType.is_equal)
                hs = {}
                for c in grp:
                    cs = slice(c * P, (c + 1) * P)
                    mvg = pools["small"].tile([P, 1, 2], F32, tag="mvg")
                    g = []
                    for j0 in range(0, FCN[1], 512):
                        ps = pools["ps_mm"].tile([P, 512], F32, tag="ps_mm")
                        nc.tensor.matmul(ps, oh[:, cs], a_sb[:, j0:j0 + 512],
                                         start=True, stop=True)
                        g.append((ps, 512))
                    _ln_stats(tc, pools, g, mvg, 0)
                    rstdg, nmrg = _group_scale(tc, pools, mvg, 1, mm_dt)
                    hs[c] = _relu_epilogue(tc, pools, g, FCN[1], mm_dt,
                                           rstdg, nmrg, 0)
                return hs

            def emit_one_layer(grp, hs, i):
                d_in, d_out = LAYER_DIMS[i]
                hTs = {c: _transpose_tiles(tc, pools, hs[c], d_in,
                                           ident_sb, mm_dt) for c in grp}
                k_tiles = d_in // P
                hs = {}
                for c in grp:
                    mvg = pools["small"].tile([P, 1, 2], F32, tag="mvg")
                    ps = pools["ps_mm"].tile([P, 512], F32, tag="ps_mm")
                    for k in range(k_tiles):
                        nc.tensor.matmul(ps[:, 0:d_out], hTs[c][:, k, :],
                                         w_sbs[i][:, k, :],
                                         start=(k == 0),
                                         stop=(k == k_tiles - 1))
                    _ln_stats(tc, pools, [(ps[:, 0:d_out], d_out)], mvg, 0)
                    rstdg, nmrg = _group_scale(tc, pools, mvg, 1, mm_dt)
                    hs[c] = _relu_epilogue(
                        tc, pools, [(ps[:, 0:d_out], d_out)], d_out,
                        mm_dt, rstdg, nmrg, 0)
                if i == len(LAYER_DIMS) - 1:
                    # head2 dot on DVE: s[:, c] = sum_i h[:, i] * hW2[i]
                    # (tensor_tensor_reduce crashes the device)
                    for c in grp:
                        scratch = pools["act"].tile([P, P], F32, tag="scr")
                        nc.vector.tensor_mul(out=scratch,
                                             in0=hs[c].bitcast(F32), in1=w2b)
                        nc.vector.reduce_sum(out=s_sb[:, c:c + 1], in_=scratch,
                                             axis=mybir.AxisListType.X)
                return hs

            # 2D wavefront over (group, stage): stage 0 = L0, stages 1..4 =
            # dense layers (stage 4 also emits the head dot). Anti-diagonal
            # emission keeps every engine fed from 2-3 groups at once and
            # shrinks the pipeline tail to a single stage.
            n_stage = 1 + len(LAYER_DIMS)
            state = {}
            for diag in range(n_groups + n_stage - 1):
                for g in range(min(diag, n_groups - 1), -1, -1):
                    s = diag - g
                    if s < 0 or s >= n_stage:
                        continue
                    if s == 0:
                        state[g] = emit_l0(groups[g])
                    else:
                        state[g] = emit_one_layer(groups[g], state[g], s - 1)

            # sigmoid once, then one strided DMA back to [R, 1]
            o_sb = w.tile([P, nchunk], F32, tag="o")
            nc.scalar.activation(out=o_sb, in_=s_sb,
                                 func=mybir.ActivationFunctionType.Sigmoid,
                                 bias=hb2_sb, scale=1.0)
            out_ap = out_d.rearrange("(c p) o -> p (c o)", p=P)
            nc.sync.dma_start(out=out_ap, in_=o_sb)

    nc.compile()
    return nc


# ---------------------------------------------------------------- host side


def _plan_routing(dom):
    """Group (element, tower) tasks by tower; pack into 8 bins of R rows."""
    col1 = COL1_TOWER[dom]
    elems = []
    for t in range(DOMAIN_NUM):
        e = np.nonzero((dom == t) | (col1 == t))[0]
        elems.append(e)
    sizes = [len(e) for e in elems]
    R = None
    for r in range(P, BATCH * 2 + P, P):
        if sum(-(-s // r) for s in sizes if s) <= 8:
            R = r
            break
    assert R is not None
    bins = []  # (tower, element_idx_array)
    for t in range(DOMAIN_NUM):
        e = elems[t]
        for b0 in range(0, len(e), R):
            bins.append((t, e[b0:b0 + R]))
    while len(bins) < 8:
        bins.append((0, np.zeros(0, np.int64)))
    assert len(bins) <= 8
    return R, bins, col1


def _fold_A(tables, W0, t):
    """A_t [80, 1024] = stack_f tables[f,:5] @ W0[t, 64f:64f+64, :]."""
    A = np.empty((N_FIELDS * DOMAIN_NUM, FCN[1]), np.float32)
    for f in range(N_FIELDS):
        A[f * DOMAIN_NUM:(f + 1) * DOMAIN_NUM] = (
            tables[f, :DOMAIN_NUM, :].astype(np.float64)
            @ W0[t, f * EMBED:(f + 1) * EMBED, :].astype(np.float64)
        ).astype(np.float32)
    return A


def _trivial_ln(mlp_bs, mlp_gs, mlp_betas, hb1, hg, hbeta):
    ok = all(np.all(np.asarray(b) == 0) for b in mlp_bs)
    ok &= all(np.all(np.asarray(g) == 1) for g in mlp_gs)
    ok &= all(np.all(np.asarray(be) == 0) for be in mlp_betas)
    ok &= np.all(np.asarray(hb1) == 0) and np.all(np.asarray(hg) == 1)
    ok &= bool(np.all(np.asarray(hbeta) == 0))
    return ok


def kernel(x, tables, mlp_Ws, mlp_bs, mlp_gs, mlp_betas, hW1, hb1, hg,
           hbeta, hW2, hb2):
    x = np.asarray(x)
    tables = np.asarray(tables, np.float32)
    mlp_Ws = [np.asarray(w, np.float32) for w in mlp_Ws]
    hW1 = np.asarray(hW1, np.float32)
    hW2 = np.asarray(hW2, np.float32)
    hb2 = np.asarray(hb2, np.float32)

    assert _trivial_ln(mlp_bs, mlp_gs, mlp_betas, hb1, hg, hbeta), \
        "non-trivial LN affine params not supported by this build"
    assert x.min() >= 0 and x.max() < DOMAIN_NUM

    dom = np.asarray(x[N_FIELDS - 1], np.int64)
    R, bins, col1 = _plan_routing(dom)

    nc = build_program(R, MM_DT)

    iota80 = (np.arange(80) % DOMAIN_NUM).astype(np.float32).reshape(80, 1)
    ident = np.eye(P, dtype=np.float32)

    tower_cache = {}

    def tower_inputs(t):
        if t not in tower_cache:
            ws = {}
            ws["A"] = _fold_A(tables, mlp_Ws[0], t)
            chain = [mlp_Ws[1][t], mlp_Ws[2][t], mlp_Ws[3][t], hW1[t]]
            for i, wmat in enumerate(chain):
                d_in, d_out = wmat.shape
                ws[f"W{i+1}"] = np.ascontiguousarray(
                    wmat.reshape(d_in // P, P, d_out).transpose(1, 0, 2))
            ws["hW2"] = hW2[t].reshape(P, 1)
            ws["hb2"] = hb2[t].reshape(1, 1).astype(np.float32)
            tower_cache[t] = ws
        return tower_cache[t]

    in_maps = []
    for t, e in bins:
        xe = np.zeros((N_FIELDS, R), np.float32)
        if len(e):
            xe[:, :len(e)] = x[:, e].astype(np.float32)
        xe = np.repeat(xe, DOMAIN_NUM, axis=0)  # [80, R], row f*5+d = x[f]
        m = {"xg": xe, "iota80": iota80, "ident": ident}
        m.update(tower_inputs(t))
        in_maps.append(m)

    res = bass_utils.run_bass_kernel_spmd(nc, in_maps, core_ids=list(range(8)),
                                          **RUN_KWARGS)
    global LAST_RESULTS
    LAST_RESULTS = res

    out = np.zeros((BATCH, 2), np.float32)
    for (t, e), r in zip(bins, res.results):
        if not len(e):
            continue
        vals = r["out"][:len(e), 0]
        m0 = dom[e] == t
        out[e[m0], 0] = vals[m0]
        m1 = col1[e] == t
        out[e[m1], 1] = vals[m1]
    return out
